# revision 2
# baseline (speedup 1.0000x reference)
import numpy as np
import jax
import jax.numpy as jnp
from jax import lax
from functools import partial

# Problem constants (hardcoded per contract)
B = 2
Hs = 48
Ws = 48
DIM = 768
NH = 6
NP = 4
DH = DIM // NH            # 128
HID = int(DIM * 0.25)     # 192
LIN = Hs * Ws             # 2304
LQ = 21 * (Hs * Ws) // 4  # 12096

# Per-batch query chunking: boundaries aligned to 96-wide rows of the
# 2H x 2W (96x96) image so the depthwise conv only needs 1-row halos.
CHUNKS = [(0, 3072), (3072, 6144), (6144, 9216), (9216, 12096)]
# ext ranges (with halo, clamped to segment boundaries)
EXTS = [(0, 3168), (2976, 6240), (6048, 9216), (9216, 12096)]
LEXT = 3264  # padded uniform ext length (34 rows of 96)


def _ln(x, g, b, eps=1e-6):
    m = jnp.mean(x, -1, keepdims=True)
    v = jnp.mean((x - m) ** 2, -1, keepdims=True)
    return (x - m) * lax.rsqrt(v + eps) * g + b


def _bilinear_sample(value, loc, Hf, Wf):
    Bq, nH = value.shape[0], value.shape[1]
    Lq = loc.shape[1]
    x = loc[..., 0] * Wf - 0.5
    y = loc[..., 1] * Hf - 0.5
    x0 = jnp.floor(x)
    y0 = jnp.floor(y)
    wx1 = x - x0
    wy1 = y - y0
    x0i = x0.astype(jnp.int32)
    y0i = y0.astype(jnp.int32)

    def tr(a):  # [B,Lq,NH,NP] -> [B,NH,Lq*NP]
        return jnp.transpose(a, (0, 2, 1, 3)).reshape(Bq, nH, Lq * NP)

    def gather(yi, xi):
        valid = ((xi >= 0) & (xi < Wf) & (yi >= 0) & (yi < Hf)).astype(value.dtype)
        idx = jnp.clip(yi, 0, Hf - 1) * Wf + jnp.clip(xi, 0, Wf - 1)
        g = jnp.take_along_axis(value, tr(idx)[..., None], axis=2)
        return g * tr(valid)[..., None]

    out = (gather(y0i, x0i) * tr((1 - wy1) * (1 - wx1))[..., None]
         + gather(y0i, x0i + 1) * tr((1 - wy1) * wx1)[..., None]
         + gather(y0i + 1, x0i) * tr(wy1 * (1 - wx1))[..., None]
         + gather(y0i + 1, x0i + 1) * tr(wy1 * wx1)[..., None])
    return out.reshape(Bq, nH, Lq, NP, DH)


def _dw(img, w, b):
    # img: [C, h, w] single image; depthwise 3x3 SAME
    y = lax.conv_general_dilated(img[None], w, (1, 1), 'SAME',
                                 dimension_numbers=('NCHW', 'OIHW', 'NCHW'),
                                 feature_group_count=img.shape[0])
    return y[0] + b[:, None, None]


def _core_forward(query, refp, feat, is_c3,
                  qn_g, qn_b, fn_g, fn_b, mn_g, mn_b, vW, vb,
                  soW, sob, awW, awb, opW, opb,
                  fc1W, fc1b, dwW, dwb, fc2W, fc2b):
    # query: [LEXT, DIM]; refp: [LEXT, 2]; feat: [LIN, DIM]; is_c3: [] f32
    Lq = query.shape[0]
    q = _ln(query, qn_g, qn_b)
    v = _ln(feat, fn_g, fn_b) @ vW + vb
    value = jnp.transpose(v.reshape(1, LIN, NH, DH), (0, 2, 1, 3))
    offs = (q @ soW + sob).reshape(1, Lq, NH, NP, 2)
    attw = jax.nn.softmax((q @ awW + awb).reshape(1, Lq, NH, NP), axis=-1)
    norm = jnp.asarray([Ws, Hs], dtype=q.dtype)
    loc = refp.reshape(1, Lq, 1, 1, 2) + offs / norm
    samp = _bilinear_sample(value, loc, Hs, Ws)
    aw = jnp.transpose(attw, (0, 2, 1, 3))[..., None]
    out = jnp.sum(samp * aw, axis=3)
    out = jnp.transpose(out, (0, 2, 1, 3)).reshape(Lq, DIM) @ opW + opb
    x = query + out
    h = _ln(x, mn_g, mn_b) @ fc1W + fc1b           # [LEXT, HID]
    hT = h.T                                        # [HID, LEXT]
    # conv variant A: whole ext block as [HID, 34, 96] image
    convA = _dw(hT.reshape(HID, LEXT // 96, 96), dwW, dwb).reshape(HID, Lq)
    # conv variant B (chunk 3): 48x48 seg + 24x24 seg + zero pad
    cB1 = _dw(hT[:, :2304].reshape(HID, 48, 48), dwW, dwb).reshape(HID, 2304)
    cB2 = _dw(hT[:, 2304:2880].reshape(HID, 24, 24), dwW, dwb).reshape(HID, 576)
    convB = jnp.concatenate([cB1, cB2, jnp.zeros((HID, Lq - 2880), hT.dtype)], axis=1)
    hc = jnp.where(is_c3 > 0.5, convB, convA).T     # [LEXT, HID]
    hc = jax.nn.gelu(hc, approximate=False)
    h2 = hc @ fc2W + fc2b
    return x + h2


_pm = None


def _get_pm():
    global _pm
    if _pm is None:
        _pm = jax.pmap(_core_forward,
                       in_axes=(0, 0, 0, 0) + (None,) * 20)
    return _pm


def kernel(query, reference_points, feat, spatial_shapes, level_start_index,
           H, W, qn_g, qn_b, fn_g, fn_b, mn_g, mn_b, vW, vb, soW, sob,
           awW, awb, opW, opb, fc1W, fc1b, dwW, dwb, fc2W, fc2b):
    query = np.asarray(query, np.float32)
    refp = np.asarray(reference_points, np.float32).reshape(B, LQ, 2)
    feat = np.asarray(feat, np.float32)

    q_sh = np.zeros((8, LEXT, DIM), np.float32)
    r_sh = np.zeros((8, LEXT, 2), np.float32)
    f_sh = np.zeros((8, LIN, DIM), np.float32)
    c3 = np.zeros((8,), np.float32)
    for c in range(8):
        b, j = c // 4, c % 4
        e0, e1 = EXTS[j]
        n = e1 - e0
        q_sh[c, :n] = query[b, e0:e1]
        r_sh[c, :n] = refp[b, e0:e1]
        f_sh[c] = feat[b]
        c3[c] = 1.0 if j == 3 else 0.0

    args = [jnp.asarray(a, jnp.float32) for a in
            (qn_g, qn_b, fn_g, fn_b, mn_g, mn_b, vW, vb, soW, sob,
             awW, awb, opW, opb, fc1W, fc1b, dwW, dwb, fc2W, fc2b)]
    out_sh = _get_pm()(jnp.asarray(q_sh), jnp.asarray(r_sh),
                       jnp.asarray(f_sh), jnp.asarray(c3), *args)
    out_sh = np.asarray(out_sh)

    out = np.zeros((B, LQ, DIM), np.float32)
    for c in range(8):
        b, j = c // 4, c % 4
        s0, s1 = CHUNKS[j]
        e0, _ = EXTS[j]
        out[b, s0:s1] = out_sh[c, s0 - e0:s1 - e0]
    return out


# revision 3
# speedup vs baseline: 1.8377x; 1.8377x over previous
import numpy as np
import jax
import jax.numpy as jnp
from jax import lax

# Problem constants (hardcoded per contract)
B = 2
Hs = 48
Ws = 48
DIM = 768
NH = 6
NP = 4
DH = DIM // NH            # 128
HID = int(DIM * 0.25)     # 192
LIN = Hs * Ws             # 2304
LQ = 21 * (Hs * Ws) // 4  # 12096

# Per-batch query chunking aligned to 96-wide rows of the 96x96 (2Hx2W)
# image so the depthwise conv needs only 1-row halos.
CHUNKS = [(0, 3072), (3072, 6144), (6144, 9216), (9216, 12096)]
EXTS = [(0, 3168), (2976, 6240), (6048, 9216), (9216, 12096)]
LEXT = 3264  # padded uniform ext length (34 rows of 96)
OWN_OFF = [0, 96, 96, 0]  # offset of own chunk within ext block


def _ln(x, g, b, eps=1e-6):
    m = jnp.mean(x, -1, keepdims=True)
    v = jnp.mean((x - m) ** 2, -1, keepdims=True)
    return (x - m) * lax.rsqrt(v + eps) * g + b


def _bilinear_sample(value, loc, Hf, Wf):
    Bq, nH = value.shape[0], value.shape[1]
    Lq = loc.shape[1]
    x = loc[..., 0] * Wf - 0.5
    y = loc[..., 1] * Hf - 0.5
    x0 = jnp.floor(x)
    y0 = jnp.floor(y)
    wx1 = x - x0
    wy1 = y - y0
    x0i = x0.astype(jnp.int32)
    y0i = y0.astype(jnp.int32)

    def tr(a):  # [B,Lq,NH,NP] -> [B,NH,Lq*NP]
        return jnp.transpose(a, (0, 2, 1, 3)).reshape(Bq, nH, Lq * NP)

    def gather(yi, xi):
        valid = ((xi >= 0) & (xi < Wf) & (yi >= 0) & (yi < Hf)).astype(value.dtype)
        idx = jnp.clip(yi, 0, Hf - 1) * Wf + jnp.clip(xi, 0, Wf - 1)
        g = jnp.take_along_axis(value, tr(idx)[..., None], axis=2)
        return g * tr(valid)[..., None]

    out = (gather(y0i, x0i) * tr((1 - wy1) * (1 - wx1))[..., None]
         + gather(y0i, x0i + 1) * tr((1 - wy1) * wx1)[..., None]
         + gather(y0i + 1, x0i) * tr(wy1 * (1 - wx1))[..., None]
         + gather(y0i + 1, x0i + 1) * tr(wy1 * wx1)[..., None])
    return out.reshape(Bq, nH, Lq, NP, DH)


def _dw(img, w, b):
    y = lax.conv_general_dilated(img[None], w, (1, 1), 'SAME',
                                 dimension_numbers=('NCHW', 'OIHW', 'NCHW'),
                                 feature_group_count=img.shape[0])
    return y[0] + b[:, None, None]


def _core_forward(query_bf, refp, feat_bf, is_c3, own_off,
                  qn_g, qn_b, fn_g, fn_b, mn_g, mn_b, vW, vb,
                  soW, sob, awW, awb, opW, opb,
                  fc1W, fc1b, dwW, dwb, fc2W, fc2b):
    # query_bf: [LEXT, DIM] bf16; refp: [LEXT, 2] f32; feat_bf: [LIN, DIM] bf16
    query = query_bf.astype(jnp.float32)
    feat = feat_bf.astype(jnp.float32)
    Lq = query.shape[0]
    q = _ln(query, qn_g, qn_b)
    v = _ln(feat, fn_g, fn_b) @ vW + vb
    value = jnp.transpose(v.reshape(1, LIN, NH, DH), (0, 2, 1, 3))
    offs = (q @ soW + sob).reshape(1, Lq, NH, NP, 2)
    attw = jax.nn.softmax((q @ awW + awb).reshape(1, Lq, NH, NP), axis=-1)
    norm = jnp.asarray([Ws, Hs], dtype=q.dtype)
    loc = refp.reshape(1, Lq, 1, 1, 2) + offs / norm
    samp = _bilinear_sample(value, loc, Hs, Ws)
    aw = jnp.transpose(attw, (0, 2, 1, 3))[..., None]
    out = jnp.sum(samp * aw, axis=3)
    out = jnp.transpose(out, (0, 2, 1, 3)).reshape(Lq, DIM) @ opW + opb
    x = query + out
    h = _ln(x, mn_g, mn_b) @ fc1W + fc1b           # [LEXT, HID]
    hT = h.T
    convA = _dw(hT.reshape(HID, LEXT // 96, 96), dwW, dwb).reshape(HID, Lq)
    cB1 = _dw(hT[:, :2304].reshape(HID, 48, 48), dwW, dwb).reshape(HID, 2304)
    cB2 = _dw(hT[:, 2304:2880].reshape(HID, 24, 24), dwW, dwb).reshape(HID, 576)
    convB = jnp.concatenate([cB1, cB2, jnp.zeros((HID, Lq - 2880), hT.dtype)], axis=1)
    hc = jnp.where(is_c3 > 0.5, convB, convA).T
    hc = jax.nn.gelu(hc, approximate=False)
    h2 = hc @ fc2W + fc2b
    delta = out + h2                                # final = query + delta
    own = lax.dynamic_slice(delta, (own_off, 0), (3072, DIM))
    return own.astype(jnp.bfloat16)


_pm = None


def _get_pm():
    global _pm
    if _pm is None:
        _pm = jax.pmap(_core_forward,
                       in_axes=(0, 0, 0, 0, 0) + (None,) * 20)
    return _pm


def kernel(query, reference_points, feat, spatial_shapes, level_start_index,
           H, W, qn_g, qn_b, fn_g, fn_b, mn_g, mn_b, vW, vb, soW, sob,
           awW, awb, opW, opb, fc1W, fc1b, dwW, dwb, fc2W, fc2b):
    query = np.asarray(query, np.float32)
    refp = np.asarray(reference_points, np.float32).reshape(B, LQ, 2)
    feat = np.asarray(feat, np.float32)

    import ml_dtypes
    bf = ml_dtypes.bfloat16
    q_sh = np.zeros((8, LEXT, DIM), bf)
    r_sh = np.zeros((8, LEXT, 2), np.float32)
    f_sh = np.zeros((8, LIN, DIM), bf)
    c3 = np.zeros((8,), np.float32)
    ooff = np.zeros((8,), np.int32)
    for c in range(8):
        b, j = c // 4, c % 4
        e0, e1 = EXTS[j]
        n = e1 - e0
        q_sh[c, :n] = query[b, e0:e1].astype(bf)
        r_sh[c, :n] = refp[b, e0:e1]
        f_sh[c] = feat[b].astype(bf)
        c3[c] = 1.0 if j == 3 else 0.0
        ooff[c] = OWN_OFF[j]

    args = [jnp.asarray(np.asarray(a, np.float32)) for a in
            (qn_g, qn_b, fn_g, fn_b, mn_g, mn_b, vW, vb, soW, sob,
             awW, awb, opW, opb, fc1W, fc1b, dwW, dwb, fc2W, fc2b)]
    out_sh = _get_pm()(jnp.asarray(q_sh), jnp.asarray(r_sh),
                       jnp.asarray(f_sh), jnp.asarray(c3),
                       jnp.asarray(ooff), *args)
    out_sh = np.asarray(out_sh).astype(np.float32)  # [8, 3072, DIM]

    out = np.empty((B, LQ, DIM), np.float32)
    for c in range(8):
        b, j = c // 4, c % 4
        s0, s1 = CHUNKS[j]
        out[b, s0:s1] = query[b, s0:s1] + out_sh[c, :s1 - s0]
    return out


# revision 4
# speedup vs baseline: 2.4672x; 1.3426x over previous
import numpy as np
import jax
import jax.numpy as jnp
from jax import lax

# Problem constants (hardcoded per contract)
B = 2
Hs = 48
Ws = 48
DIM = 768
NH = 6
NP = 4
DH = DIM // NH            # 128
HID = int(DIM * 0.25)     # 192
LIN = Hs * Ws             # 2304
LQ = 21 * (Hs * Ws) // 4  # 12096

# Per-batch query chunking aligned to 96-wide rows of the 96x96 (2Hx2W)
# image so the depthwise conv needs only 1-row halos.
CHUNKS = [(0, 3072), (3072, 6144), (6144, 9216), (9216, 12096)]
EXTS = [(0, 3168), (2976, 6240), (6048, 9216), (9216, 12096)]
LEXT = 3264  # padded uniform ext length (34 rows of 96)
OWN_OFF = [0, 96, 96, 0]  # offset of own chunk within ext block

_GROUPS = [[0, 1, 2, 3], [4, 5, 6, 7]]


def _ln(x, g, b, eps=1e-6):
    m = jnp.mean(x, -1, keepdims=True)
    v = jnp.mean((x - m) ** 2, -1, keepdims=True)
    return (x - m) * lax.rsqrt(v + eps) * g + b


def _bilinear_sample(value, loc, Hf, Wf):
    Bq, nH = value.shape[0], value.shape[1]
    Lq = loc.shape[1]
    x = loc[..., 0] * Wf - 0.5
    y = loc[..., 1] * Hf - 0.5
    x0 = jnp.floor(x)
    y0 = jnp.floor(y)
    wx1 = x - x0
    wy1 = y - y0
    x0i = x0.astype(jnp.int32)
    y0i = y0.astype(jnp.int32)

    def tr(a):  # [B,Lq,NH,NP] -> [B,NH,Lq*NP]
        return jnp.transpose(a, (0, 2, 1, 3)).reshape(Bq, nH, Lq * NP)

    def gather(yi, xi):
        valid = ((xi >= 0) & (xi < Wf) & (yi >= 0) & (yi < Hf)).astype(value.dtype)
        idx = jnp.clip(yi, 0, Hf - 1) * Wf + jnp.clip(xi, 0, Wf - 1)
        g = jnp.take_along_axis(value, tr(idx)[..., None], axis=2)
        return g * tr(valid)[..., None]

    out = (gather(y0i, x0i) * tr((1 - wy1) * (1 - wx1))[..., None]
         + gather(y0i, x0i + 1) * tr((1 - wy1) * wx1)[..., None]
         + gather(y0i + 1, x0i) * tr(wy1 * (1 - wx1))[..., None]
         + gather(y0i + 1, x0i + 1) * tr(wy1 * wx1)[..., None])
    return out.reshape(Bq, nH, Lq, NP, DH)


def _dw(img, w, b):
    y = lax.conv_general_dilated(img[None], w, (1, 1), 'SAME',
                                 dimension_numbers=('NCHW', 'OIHW', 'NCHW'),
                                 feature_group_count=img.shape[0])
    return y[0] + b[:, None, None]


def _gath(part):
    g = lax.all_gather(part, 'i', axis_index_groups=None)  # placeholder
    return g


def _core_forward(query_h, refp, feat_part, wpk, is_c3, own_off,
                  qn_g, qn_b, fn_g, fn_b, mn_g, mn_b, vb, sob, awb,
                  opb, fc1b, dwW, dwb, fc2b):
    # query_h: [LEXT, DIM] f16; feat_part: [LIN//4, DIM] f16 (batch-group shard)
    # wpk: [WPK] f16 — per-device shard of packed big weights
    query = query_h.astype(jnp.float32)
    feat = lax.all_gather(feat_part, 'i', axis_index_groups=_GROUPS)
    feat = feat.reshape(LIN, DIM).astype(jnp.float32)
    wall = lax.all_gather(wpk, 'i').reshape(-1)  # full packed weights, f16
    o = 0

    def take(n, shape):
        nonlocal o
        w = wall[o:o + n].reshape(shape).astype(jnp.float32)
        o += n
        return w
    vW = take(DIM * DIM, (DIM, DIM))
    opW = take(DIM * DIM, (DIM, DIM))
    fc1W = take(DIM * HID, (DIM, HID))
    fc2W = take(HID * DIM, (HID, DIM))
    soW = take(DIM * NH * NP * 2, (DIM, NH * NP * 2))
    awW = take(DIM * NH * NP, (DIM, NH * NP))

    Lq = query.shape[0]
    q = _ln(query, qn_g, qn_b)
    v = _ln(feat, fn_g, fn_b) @ vW + vb
    value = jnp.transpose(v.reshape(1, LIN, NH, DH), (0, 2, 1, 3))
    offs = (q @ soW + sob).reshape(1, Lq, NH, NP, 2)
    attw = jax.nn.softmax((q @ awW + awb).reshape(1, Lq, NH, NP), axis=-1)
    norm = jnp.asarray([Ws, Hs], dtype=q.dtype)
    loc = refp.reshape(1, Lq, 1, 1, 2) + offs / norm
    samp = _bilinear_sample(value, loc, Hs, Ws)
    aw = jnp.transpose(attw, (0, 2, 1, 3))[..., None]
    out = jnp.sum(samp * aw, axis=3)
    out = jnp.transpose(out, (0, 2, 1, 3)).reshape(Lq, DIM) @ opW + opb
    x = query + out
    h = _ln(x, mn_g, mn_b) @ fc1W + fc1b           # [LEXT, HID]
    hT = h.T
    convA = _dw(hT.reshape(HID, LEXT // 96, 96), dwW, dwb).reshape(HID, Lq)
    cB1 = _dw(hT[:, :2304].reshape(HID, 48, 48), dwW, dwb).reshape(HID, 2304)
    cB2 = _dw(hT[:, 2304:2880].reshape(HID, 24, 24), dwW, dwb).reshape(HID, 576)
    convB = jnp.concatenate([cB1, cB2, jnp.zeros((HID, Lq - 2880), hT.dtype)], axis=1)
    hc = jnp.where(is_c3 > 0.5, convB, convA).T
    hc = jax.nn.gelu(hc, approximate=False)
    h2 = hc @ fc2W + fc2b
    delta = out + h2                                # final = query + delta
    own = lax.dynamic_slice(delta, (own_off, 0), (3072, DIM))
    return own.astype(jnp.float16)


_pm = None


def _get_pm():
    global _pm
    if _pm is None:
        _pm = jax.pmap(_core_forward, axis_name='i',
                       in_axes=(0, 0, 0, 0, 0, 0) + (None,) * 14)
    return _pm


def kernel(query, reference_points, feat, spatial_shapes, level_start_index,
           H, W, qn_g, qn_b, fn_g, fn_b, mn_g, mn_b, vW, vb, soW, sob,
           awW, awb, opW, opb, fc1W, fc1b, dwW, dwb, fc2W, fc2b):
    query = np.asarray(query, np.float32)
    refp = np.asarray(reference_points, np.float32).reshape(B, LQ, 2)
    feat = np.asarray(feat, np.float32)

    f16 = np.float16
    q_sh = np.zeros((8, LEXT, DIM), f16)
    r_sh = np.zeros((8, LEXT, 2), np.float32)
    f_sh = np.zeros((8, LIN // 4, DIM), f16)
    c3 = np.zeros((8,), np.float32)
    ooff = np.zeros((8,), np.int32)
    for c in range(8):
        b, j = c // 4, c % 4
        e0, e1 = EXTS[j]
        n = e1 - e0
        q_sh[c, :n] = query[b, e0:e1].astype(f16)
        r_sh[c, :n] = refp[b, e0:e1]
        f_sh[c] = feat[b, j * (LIN // 4):(j + 1) * (LIN // 4)].astype(f16)
        c3[c] = 1.0 if j == 3 else 0.0
        ooff[c] = OWN_OFF[j]

    wpacked = np.concatenate([
        np.asarray(w, np.float32).reshape(-1) for w in
        (vW, opW, fc1W, fc2W, soW, awW)]).astype(f16)
    npk = wpacked.size
    pad = (-npk) % 8
    if pad:
        wpacked = np.concatenate([wpacked, np.zeros((pad,), f16)])
    w_sh = wpacked.reshape(8, -1)

    small = [jnp.asarray(np.asarray(a, np.float32)) for a in
             (qn_g, qn_b, fn_g, fn_b, mn_g, mn_b, vb, sob, awb,
              opb, fc1b, dwW, dwb, fc2b)]
    out_sh = _get_pm()(jnp.asarray(q_sh), jnp.asarray(r_sh),
                       jnp.asarray(f_sh), jnp.asarray(w_sh),
                       jnp.asarray(c3), jnp.asarray(ooff), *small)
    out_sh = np.asarray(out_sh).astype(np.float32)  # [8, 3072, DIM]

    out = np.empty((B, LQ, DIM), np.float32)
    for c in range(8):
        b, j = c // 4, c % 4
        s0, s1 = CHUNKS[j]
        out[b, s0:s1] = query[b, s0:s1] + out_sh[c, :s1 - s0]
    return out
